# revision 4
# baseline (speedup 1.0000x reference)
"""Batched dynamic-weight depthwise cross-correlation on 8 trn2 NeuronCores.

out[b, y, x, c] = sum_{i,j} search[b, y+i, x+j, c] * template[b, i, j, c]
search: (128, 31, 31, 256) f32, template: (128, 7, 7, 256) f32 -> (128, 25, 25, 256) f32

Sharding: pure data parallel over batch (16 per core).

Host pre-pass: transpose both tensors to channel-partition-major layout and
cast search to bf16 — removes every on-device transpose.  Per-core unit =
one (batch, channel-group-of-128); per unit the 49 taps are split:

  - PE taps: diag(t[c,ij]) @ shifted bf16 search windows accumulated in PSUM
    (bf16 streams 1 cyc/row with no even/256-column constraint, so windows
    are exact 25-wide; psum split 13/12 rows across two banks).
  - DVE taps: two interleaved scalar_tensor_tensor MAC chains in SBUF bf16.
  - Pool (GPSIMD) taps: tensor_scalar multiply into a scratch then
    TensorTensor add into its own accumulator (STT is illegal on Pool).
  - Diags are built on ACT/Pool/DVE from a bf16 identity and f32 scalars.
  - ACT evacuates the PSUM partial to bf16; the partial streams (psum evac,
    DVE chains, pool) leave in ONE DMA per unit; the host sums them in f32.
"""

import numpy as np
import ml_dtypes

import concourse.bacc as bacc
import concourse.bass as bass
import concourse.tile as tile
from concourse import mybir
from concourse.bass_utils import run_bass_kernel_spmd

K = 7
X = 31
O = 25  # X - K + 1
OP = O * O  # 625
SP = X * X  # 961
B = 128
C = 256
N_CORES = 8
BL = B // N_CORES  # 16 batches per core
CG = 2  # channel groups of 128
F32 = mybir.dt.float32
BF16 = mybir.dt.bfloat16

# ---- tap split across engines (total 49) ----
N_PE = 32
N_DVE = 14  # split into N_CHAINS interleaved chains
N_POOL = 3  # via tensor_scalar mult + TT add (2 instrs each)
N_CHAINS = 3

# ---- diag builders for the PE taps (total N_PE): [ACT, DVE, POOL] ----
DG_ACT = 24
DG_DVE = 0
DG_POOL = 8

YSPLIT = 13  # psum bank split: pa = 13*25, pb = 12*25

DIAG_BUFS = 12
S_BUFS = 3
OUT_BUFS = 3
SCR_BUFS = 2
PS_BUFS = 2

MULT = mybir.AluOpType.mult
ADD = mybir.AluOpType.add


def _gpsimd_tt(nc, out, in0, in1, op):
    """InstTensorTensor on the Pool engine (no bass wrapper exists)."""
    g = nc.gpsimd
    return g.add_instruction(
        mybir.InstTensorTensor(
            name=nc.get_next_instruction_name(),
            op=op,
            ins=[g.lower_ap(in0), g.lower_ap(in1)],
            outs=[g.lower_ap(out)],
        )
    )


def _build_bass(n_pe=N_PE, n_dve=N_DVE, n_pool=N_POOL, dg=(DG_ACT, DG_DVE, DG_POOL),
                n_chains=N_CHAINS):
    assert n_pe + n_dve + n_pool == K * K
    dg_act, dg_dve, dg_pool = dg
    assert dg_act + dg_dve + dg_pool == n_pe
    n_streams = 1 + n_chains + (1 if n_pool else 0)

    nc = bacc.Bacc("TRN2", target_bir_lowering=False, debug=False)

    search = nc.dram_tensor("search", [BL, 128, CG, SP], BF16, kind="ExternalInput")
    template = nc.dram_tensor("template", [BL, 128, CG, K * K], F32, kind="ExternalInput")
    eye = nc.dram_tensor("eye", [128, 128], BF16, kind="ExternalInput")
    out = nc.dram_tensor("out", [BL, CG, 128, n_streams, OP], BF16, kind="ExternalOutput")

    taps = [(i, j) for i in range(K) for j in range(K)]
    pe_taps = taps[:n_pe]
    dve_taps = taps[n_pe : n_pe + n_dve]
    pool_taps = taps[n_pe + n_dve :]

    with tile.TileContext(nc) as tc:
        with (
            tc.tile_pool(name="singles", bufs=1) as singles,
            tc.tile_pool(name="p_s", bufs=S_BUFS) as p_s,
            tc.tile_pool(name="p_diag", bufs=DIAG_BUFS) as p_diag,
            tc.tile_pool(name="p_out", bufs=OUT_BUFS) as p_out,
            tc.tile_pool(name="p_scr", bufs=SCR_BUFS) as p_scr,
            tc.tile_pool(name="ps_a", bufs=PS_BUFS, space="PSUM") as ps_a,
            tc.tile_pool(name="ps_b", bufs=PS_BUFS, space="PSUM") as ps_b,
        ):
            eye_sb = singles.tile([128, 128], BF16, name="eye_sb")
            nc.sync.dma_start(out=eye_sb[:], in_=eye.ap()[:, :])
            # all templates for the core, one DMA: [c, b, cg, ij] f32
            t_all = singles.tile([128, BL, CG, K * K], F32, name="t_all")
            nc.sync.dma_start(
                out=t_all[:], in_=template.ap()[:, :, :, :].rearrange("b c g k -> c b g k")
            )

            for b in range(BL):
                with tc.high_priority(offset=40):
                    s_nat = p_s.tile([128, CG, SP], BF16, name="s_nat")
                    nc.sync.dma_start(out=s_nat[:], in_=search.ap()[b, :, :, :])

                for cg in range(CG):
                    s3 = s_nat[:, cg, :].rearrange("p (y x) -> p y x", x=X)

                    def t_ap(ij):
                        return t_all[:, b, cg, ij : ij + 1]

                    # ---- diag builds (hoisted in priority so PE never stalls)
                    diags = []
                    with tc.high_priority(offset=60):
                        for n, (i, j) in enumerate(pe_taps):
                            ij = i * K + j
                            diag = p_diag.tile([128, 128], BF16, name="diag", tag="diag")
                            if n < dg_act:
                                nc.scalar.mul(out=diag[:], in_=eye_sb[:], mul=t_ap(ij))
                            elif n < dg_act + dg_dve:
                                nc.vector.tensor_scalar_mul(
                                    out=diag[:], in0=eye_sb[:], scalar1=t_ap(ij)
                                )
                            else:
                                nc.gpsimd.tensor_scalar_mul(
                                    out=diag[:], in0=eye_sb[:], scalar1=t_ap(ij)
                                )
                            diags.append(diag)

                    # ---- PE taps: diag @ window -> psum accumulate
                    pa = ps_a.tile([128, YSPLIT * O], F32, name="pa", tag="pa")
                    pb = ps_b.tile([128, (O - YSPLIT) * O], F32, name="pb", tag="pb")
                    for n, (i, j) in enumerate(pe_taps):
                        first = n == 0
                        last = n == n_pe - 1
                        nc.tensor.matmul(
                            pa[:],
                            diags[n][:],
                            s3[:, i : i + YSPLIT, j : j + O],
                            start=first,
                            stop=last,
                        )
                        nc.tensor.matmul(
                            pb[:],
                            diags[n][:],
                            s3[:, i + YSPLIT : i + O, j : j + O],
                            start=first,
                            stop=last,
                        )

                    # ---- output tile: bf16 streams summed on the host
                    o_t = p_out.tile([128, n_streams, OP], BF16, name="o_t", tag="o_t")

                    # ACT evacuates the PE psum partial
                    nc.scalar.copy(out=o_t[:, 0, 0 : YSPLIT * O], in_=pa[:])
                    nc.scalar.copy(out=o_t[:, 0, YSPLIT * O : OP], in_=pb[:])

                    # ---- DVE taps: interleaved MAC chains
                    accs = [
                        o_t[:, 1 + ch, :].rearrange("p (y x) -> p y x", x=O)
                        for ch in range(n_chains)
                    ]
                    n_chain = [0] * n_chains
                    for n, (i, j) in enumerate(dve_taps):
                        ij = i * K + j
                        ch = n % n_chains
                        win = s3[:, i : i + O, j : j + O]
                        if n_chain[ch] == 0:
                            nc.vector.tensor_scalar_mul(
                                out=accs[ch][:], in0=win, scalar1=t_ap(ij)
                            )
                        else:
                            nc.vector.scalar_tensor_tensor(
                                out=accs[ch][:],
                                in0=win,
                                scalar=t_ap(ij),
                                in1=accs[ch][:],
                                op0=MULT,
                                op1=ADD,
                            )
                        n_chain[ch] += 1

                    # ---- Pool taps: tensor_scalar mult to scratch + TT add
                    if pool_taps:
                        accp = o_t[:, 1 + n_chains, :]
                        for n, (i, j) in enumerate(pool_taps):
                            ij = i * K + j
                            win = s3[:, i : i + O, j : j + O]
                            if n == 0:
                                nc.gpsimd.tensor_scalar_mul(
                                    out=accp.rearrange("p (y x) -> p y x", x=O),
                                    in0=win,
                                    scalar1=t_ap(ij),
                                )
                            else:
                                scr = p_scr.tile([128, OP], BF16, name="scr", tag="scr")
                                nc.gpsimd.tensor_scalar_mul(
                                    out=scr[:].rearrange("p (y x) -> p y x", x=O),
                                    in0=win,
                                    scalar1=t_ap(ij),
                                )
                                _gpsimd_tt(nc, accp, accp, scr[:], ADD)

                    # ---- one DMA out per unit
                    nc.sync.dma_start(out=out.ap()[b, cg, :, :, :], in_=o_t[:])
    nc.compile()
    return nc


_NC_CACHE = None


def _get_nc():
    global _NC_CACHE
    if _NC_CACHE is None:
        _NC_CACHE = _build_bass()
    return _NC_CACHE


def _prep_inputs(search: np.ndarray, template: np.ndarray):
    search = np.ascontiguousarray(np.asarray(search), dtype=np.float32)
    template = np.ascontiguousarray(np.asarray(template), dtype=np.float32)
    # [B, y, x, c] -> [B, c_part(128), cg(2), y*x]
    s_t = (
        search.transpose(0, 3, 1, 2)
        .reshape(B, CG, 128, SP)
        .transpose(0, 2, 1, 3)
        .astype(ml_dtypes.bfloat16)
    )
    t_t = (
        template.transpose(0, 3, 1, 2)
        .reshape(B, CG, 128, K * K)
        .transpose(0, 2, 1, 3)
        .astype(np.float32)
    )
    s_t = np.ascontiguousarray(s_t)
    t_t = np.ascontiguousarray(t_t)
    return s_t, t_t


def _run(search: np.ndarray, template: np.ndarray, **spmd_kwargs):
    nc = _get_nc()
    s_t, t_t = _prep_inputs(search, template)
    eye = np.eye(128, dtype=ml_dtypes.bfloat16)
    in_maps = [
        {
            "search": s_t[c * BL : (c + 1) * BL],
            "template": t_t[c * BL : (c + 1) * BL],
            "eye": eye,
        }
        for c in range(N_CORES)
    ]
    res = run_bass_kernel_spmd(nc, in_maps, core_ids=list(range(N_CORES)), **spmd_kwargs)
    # out: [BL, CG, 128, n_streams, 625] bf16 per core -> sum streams, reassemble
    outs = []
    for r in res.results:
        o = np.asarray(r["out"]).astype(np.float32).sum(axis=3)  # [BL, CG, 128, 625]
        o = (
            o.reshape(BL, CG, 128, O, O)
            .transpose(0, 3, 4, 1, 2)
            .reshape(BL, O, O, C)
        )
        outs.append(o)
    out = np.concatenate(outs, axis=0)
    return out, res


def kernel(search: np.ndarray, template: np.ndarray) -> np.ndarray:
    out, _ = _run(search, template)
    return out


# revision 10
# speedup vs baseline: 1.1156x; 1.1156x over previous
"""Batched dynamic-weight depthwise cross-correlation on 8 trn2 NeuronCores.

out[b, y, x, c] = sum_{i,j} search[b, y+i, x+j, c] * template[b, i, j, c]
search: (128, 31, 31, 256) f32, template: (128, 7, 7, 256) f32 -> (128, 25, 25, 256) f32

Sharding: pure data parallel over batch (16 per core).

Host pre-pass: transpose both tensors to channel-partition-major layout and
cast search to bf16 — removes every on-device transpose.  Per-core unit =
one (batch, channel-group-of-128); per unit the 49 taps are split:

  - PE taps: diag(t[c,ij]) @ shifted bf16 search windows accumulated in PSUM
    (bf16 streams 1 cyc/row with no even/256-column constraint, so windows
    are exact 25-wide; psum split 13/12 rows across two banks).
  - DVE taps: two interleaved scalar_tensor_tensor MAC chains in SBUF bf16.
  - Pool (GPSIMD) taps: tensor_scalar multiply into a scratch then
    TensorTensor add into its own accumulator (STT is illegal on Pool).
  - Diags are built on ACT/Pool/DVE from a bf16 identity and f32 scalars.
  - ACT evacuates the PSUM partial to bf16; the partial streams (psum evac,
    DVE chains, pool) leave in ONE DMA per unit; the host sums them in f32.
"""

import numpy as np
import ml_dtypes

import concourse.bacc as bacc
import concourse.bass as bass
import concourse.tile as tile
from concourse import mybir
from concourse.bass_utils import run_bass_kernel_spmd

K = 7
X = 31
O = 25  # X - K + 1
OP = O * O  # 625
SP = X * X  # 961
B = 128
C = 256
N_CORES = 8
BL = B // N_CORES  # 16 batches per core
CG = 2  # channel groups of 128
F32 = mybir.dt.float32
BF16 = mybir.dt.bfloat16

# ---- tap split across engines (total 49) ----
N_PE = 28
N_DVE = 11  # STT taps, split into N_CHAINS interleaved chains
N_POOL = 0  # legacy pool mult+TT-add taps
N_CHAINS = 3
M_DVE = 5   # single-multiply taps on DVE, own output stream each
M_POOL = 5  # single-multiply taps on Pool, own output stream each
M_ACT = 0   # single-multiply taps on ACT, own output stream each

# ---- diag builders for the PE taps (total N_PE): [ACT, DVE, POOL] ----
DG_ACT = 19
DG_DVE = 0
DG_POOL = 9

YSPLIT = 13  # psum bank split: pa = 13*25, pb = 12*25

DIAG_BUFS = 20
S_BUFS = 3
OUT_BUFS = 4
SCR_BUFS = 2
PS_BUFS = 3

MULT = mybir.AluOpType.mult
ADD = mybir.AluOpType.add


def _gpsimd_tt(nc, out, in0, in1, op):
    """InstTensorTensor on the Pool engine (no bass wrapper exists)."""
    g = nc.gpsimd
    return g.add_instruction(
        mybir.InstTensorTensor(
            name=nc.get_next_instruction_name(),
            op=op,
            ins=[g.lower_ap(in0), g.lower_ap(in1)],
            outs=[g.lower_ap(out)],
        )
    )


def _build_bass(n_pe=N_PE, n_dve=N_DVE, n_pool=N_POOL, dg=(DG_ACT, DG_DVE, DG_POOL),
                n_chains=N_CHAINS, m_dve=M_DVE, m_pool=M_POOL, m_act=M_ACT):
    # taps: n_pe diag-matmul taps; n_dve STT-chain taps on DVE (n_chains
    # chains); m_dve/m_pool single-multiply taps streaming their own bf16
    # partial (host sums); n_pool legacy pool mult+add taps.
    assert n_pe + n_dve + n_pool + m_dve + m_pool + m_act == K * K
    dg_act, dg_dve, dg_pool = dg
    assert dg_act + dg_dve + dg_pool == n_pe
    n_streams = 1 + n_chains + (1 if n_pool else 0) + m_dve + m_pool + m_act

    nc = bacc.Bacc("TRN2", target_bir_lowering=False, debug=False)

    search = nc.dram_tensor("search", [BL, 128, CG, SP], BF16, kind="ExternalInput")
    template = nc.dram_tensor("template", [BL, 128, CG, K * K], F32, kind="ExternalInput")
    eye = nc.dram_tensor("eye", [128, 128], BF16, kind="ExternalInput")
    out = nc.dram_tensor("out", [BL, CG, 128, n_streams, OP], BF16, kind="ExternalOutput")

    taps = [(i, j) for i in range(K) for j in range(K)]
    pe_taps = taps[:n_pe]
    dve_taps = taps[n_pe : n_pe + n_dve]
    pool_taps = taps[n_pe + n_dve : n_pe + n_dve + n_pool]
    mul_dve_taps = taps[n_pe + n_dve + n_pool : n_pe + n_dve + n_pool + m_dve]
    mul_pool_taps = taps[
        n_pe + n_dve + n_pool + m_dve : n_pe + n_dve + n_pool + m_dve + m_pool
    ]
    mul_act_taps = taps[n_pe + n_dve + n_pool + m_dve + m_pool :]

    with tile.TileContext(nc) as tc:
        with (
            tc.tile_pool(name="singles", bufs=1) as singles,
            tc.tile_pool(name="p_s", bufs=S_BUFS) as p_s,
            tc.tile_pool(name="p_diag", bufs=DIAG_BUFS) as p_diag,
            tc.tile_pool(name="p_out", bufs=OUT_BUFS) as p_out,
            tc.tile_pool(name="p_scr", bufs=SCR_BUFS) as p_scr,
            tc.tile_pool(name="ps_a", bufs=PS_BUFS, space="PSUM") as ps_a,
            tc.tile_pool(name="ps_b", bufs=PS_BUFS, space="PSUM") as ps_b,
        ):
            eye_sb = singles.tile([128, 128], BF16, name="eye_sb")
            nc.sync.dma_start(out=eye_sb[:], in_=eye.ap()[:, :])
            # all templates for the core, one DMA: [c, b, cg, ij] f32
            t_all = singles.tile([128, BL, CG, K * K], F32, name="t_all")
            nc.sync.dma_start(
                out=t_all[:], in_=template.ap()[:, :, :, :].rearrange("b c g k -> c b g k")
            )

            for b in range(BL):
                with tc.high_priority(offset=40):
                    s_nat = p_s.tile([128, CG, SP], BF16, name="s_nat")
                    nc.sync.dma_start(out=s_nat[:], in_=search.ap()[b, :, :, :])

                for cg in range(CG):
                    s3 = s_nat[:, cg, :].rearrange("p (y x) -> p y x", x=X)

                    def t_ap(ij):
                        return t_all[:, b, cg, ij : ij + 1]

                    # ---- diag builds (hoisted in priority so PE never stalls)
                    diags = []
                    with tc.high_priority(offset=60):
                        for n, (i, j) in enumerate(pe_taps):
                            ij = i * K + j
                            diag = p_diag.tile([128, 128], BF16, name="diag", tag="diag")
                            if n < dg_act:
                                nc.scalar.mul(out=diag[:], in_=eye_sb[:], mul=t_ap(ij))
                            elif n < dg_act + dg_dve:
                                nc.vector.tensor_scalar_mul(
                                    out=diag[:], in0=eye_sb[:], scalar1=t_ap(ij)
                                )
                            else:
                                nc.gpsimd.tensor_scalar_mul(
                                    out=diag[:], in0=eye_sb[:], scalar1=t_ap(ij)
                                )
                            diags.append(diag)

                    # ---- PE taps: diag @ window -> psum accumulate
                    pa = ps_a.tile([128, YSPLIT * O], F32, name="pa", tag="pa")
                    pb = ps_b.tile([128, (O - YSPLIT) * O], F32, name="pb", tag="pb")
                    for n, (i, j) in enumerate(pe_taps):
                        first = n == 0
                        last = n == n_pe - 1
                        nc.tensor.matmul(
                            pa[:],
                            diags[n][:],
                            s3[:, i : i + YSPLIT, j : j + O],
                            start=first,
                            stop=last,
                        )
                        nc.tensor.matmul(
                            pb[:],
                            diags[n][:],
                            s3[:, i + YSPLIT : i + O, j : j + O],
                            start=first,
                            stop=last,
                        )

                    # ---- output tile: bf16 streams summed on the host
                    o_t = p_out.tile([128, n_streams, OP], BF16, name="o_t", tag="o_t")

                    # ACT evacuates the PE psum partial
                    nc.scalar.copy(out=o_t[:, 0, 0 : YSPLIT * O], in_=pa[:])
                    nc.scalar.copy(out=o_t[:, 0, YSPLIT * O : OP], in_=pb[:])

                    # ---- DVE taps: interleaved MAC chains
                    accs = [
                        o_t[:, 1 + ch, :].rearrange("p (y x) -> p y x", x=O)
                        for ch in range(n_chains)
                    ]
                    n_chain = [0] * n_chains
                    for n, (i, j) in enumerate(dve_taps):
                        ij = i * K + j
                        ch = n % n_chains
                        win = s3[:, i : i + O, j : j + O]
                        if n_chain[ch] == 0:
                            nc.vector.tensor_scalar_mul(
                                out=accs[ch][:], in0=win, scalar1=t_ap(ij)
                            )
                        else:
                            nc.vector.scalar_tensor_tensor(
                                out=accs[ch][:],
                                in0=win,
                                scalar=t_ap(ij),
                                in1=accs[ch][:],
                                op0=MULT,
                                op1=ADD,
                            )
                        n_chain[ch] += 1

                    # ---- Pool taps: tensor_scalar mult to scratch + TT add
                    if pool_taps:
                        accp = o_t[:, 1 + n_chains, :]
                        for n, (i, j) in enumerate(pool_taps):
                            ij = i * K + j
                            win = s3[:, i : i + O, j : j + O]
                            if n == 0:
                                nc.gpsimd.tensor_scalar_mul(
                                    out=accp.rearrange("p (y x) -> p y x", x=O),
                                    in0=win,
                                    scalar1=t_ap(ij),
                                )
                            else:
                                scr = p_scr.tile([128, OP], BF16, name="scr", tag="scr")
                                nc.gpsimd.tensor_scalar_mul(
                                    out=scr[:].rearrange("p (y x) -> p y x", x=O),
                                    in0=win,
                                    scalar1=t_ap(ij),
                                )
                                _gpsimd_tt(nc, accp, accp, scr[:], ADD)


                    # ---- single-multiply taps: own bf16 stream each
                    s_base = 1 + n_chains + (1 if n_pool else 0)
                    for n, (i, j) in enumerate(mul_dve_taps):
                        ij = i * K + j
                        win = s3[:, i : i + O, j : j + O]
                        nc.vector.tensor_scalar_mul(
                            out=o_t[:, s_base + n, :].rearrange(
                                "p (y x) -> p y x", x=O
                            ),
                            in0=win,
                            scalar1=t_ap(ij),
                        )
                    for n, (i, j) in enumerate(mul_pool_taps):
                        ij = i * K + j
                        win = s3[:, i : i + O, j : j + O]
                        nc.gpsimd.tensor_scalar_mul(
                            out=o_t[:, s_base + m_dve + n, :].rearrange(
                                "p (y x) -> p y x", x=O
                            ),
                            in0=win,
                            scalar1=t_ap(ij),
                        )
                    for n, (i, j) in enumerate(mul_act_taps):
                        ij = i * K + j
                        win = s3[:, i : i + O, j : j + O]
                        nc.scalar.mul(
                            out=o_t[:, s_base + m_dve + m_pool + n, :].rearrange(
                                "p (y x) -> p y x", x=O
                            ),
                            in_=win,
                            mul=t_ap(ij),
                        )

                    # ---- one DMA out per unit
                    nc.sync.dma_start(out=out.ap()[b, cg, :, :, :], in_=o_t[:])
    nc.compile()
    return nc


_NC_CACHE = None


def _get_nc():
    global _NC_CACHE
    if _NC_CACHE is None:
        _NC_CACHE = _build_bass()
    return _NC_CACHE


def _prep_inputs(search: np.ndarray, template: np.ndarray):
    search = np.ascontiguousarray(np.asarray(search), dtype=np.float32)
    template = np.ascontiguousarray(np.asarray(template), dtype=np.float32)
    # [B, y, x, c] -> [B, c_part(128), cg(2), y*x]
    s_t = (
        search.transpose(0, 3, 1, 2)
        .reshape(B, CG, 128, SP)
        .transpose(0, 2, 1, 3)
        .astype(ml_dtypes.bfloat16)
    )
    t_t = (
        template.transpose(0, 3, 1, 2)
        .reshape(B, CG, 128, K * K)
        .transpose(0, 2, 1, 3)
        .astype(np.float32)
    )
    s_t = np.ascontiguousarray(s_t)
    t_t = np.ascontiguousarray(t_t)
    return s_t, t_t


def _run(search: np.ndarray, template: np.ndarray, **spmd_kwargs):
    nc = _get_nc()
    s_t, t_t = _prep_inputs(search, template)
    eye = np.eye(128, dtype=ml_dtypes.bfloat16)
    in_maps = [
        {
            "search": s_t[c * BL : (c + 1) * BL],
            "template": t_t[c * BL : (c + 1) * BL],
            "eye": eye,
        }
        for c in range(N_CORES)
    ]
    res = run_bass_kernel_spmd(nc, in_maps, core_ids=list(range(N_CORES)), **spmd_kwargs)
    # out: [BL, CG, 128, n_streams, 625] bf16 per core -> sum streams, reassemble
    outs = []
    for r in res.results:
        o = np.asarray(r["out"]).astype(np.float32).sum(axis=3)  # [BL, CG, 128, 625]
        o = (
            o.reshape(BL, CG, 128, O, O)
            .transpose(0, 3, 4, 1, 2)
            .reshape(BL, O, O, C)
        )
        outs.append(o)
    out = np.concatenate(outs, axis=0)
    return out, res


def kernel(search: np.ndarray, template: np.ndarray) -> np.ndarray:
    out, _ = _run(search, template)
    return out
